# revision 9
# baseline (speedup 1.0000x reference)
"""Trainium2 Bass kernel: multi-head causal attention with RoPE (LLaMA-style).

Problem: y = Attention(x) with B=2, S=2048, D=2048, H=16 heads, HD=128,
torch-Linear convention (y = x @ W.T), interleaved-rope, additive mask.

Sharding (8 NeuronCores): batch (2) x head-groups (4) grid.  Core c handles
batch b = c // 4 and heads 4g..4g+3 where g = c % 4 (tensor parallel:
wq/wk/wv column-parallel, wo row-parallel).  Each core returns a partial
y contribution [S, D]; the host sums the 4 partials per batch.

Layout strategy (no on-chip transposes anywhere):
  - Host pre-transposes: xT [D,S], wqT/wkT/wvT [D,E], woT [E,D].
  - Q^T,K^T computed directly in [hd, s] layout (hd = partitions) with the
    head-dim DEINTERLEAVED (rows 0-63 = even/"re" dims, 64-127 = odd/"im")
    by permuting wq/wk columns on the host; RoPE is then plain 64-partition
    elementwise ops.  The permutation is invisible to Q.K^T contraction.
  - scores are computed TRANSPOSED [sk, sq] so softmax-denominators come
    from a ones-matmul (column sums) and exp(scores)^T feeds the PV matmul
    directly as the moving operand: P^T never materializes.
  - attention out falls out as out^T [hd, sq] = exactly the stationary
    layout the wo row-parallel matmul wants.
Matmul inputs are bf16 (fp32 PSUM accumulation); softmax runs in fp32.
"""

import math
from contextlib import ExitStack

import numpy as np
import ml_dtypes

P = 128          # partitions / head dim
CW = 512         # s-chunk width (one PSUM bank of fp32)

_built_cache = {}


def _build(*, S, D, E, mask_mode):
    """Build + compile the SPMD Bass program for one core's shard.

    S: sequence length, D: model dim, E: head-columns per core (nH*128).
    mask_mode: 'causal' (use diag block + skip upper triangle),
               'none' (no mask, full attention),
               'generic' (arbitrary additive mask, applied everywhere).
    """
    import concourse.bacc as bacc
    import concourse.mybir as mybir
    import concourse.tile as tile

    f32 = mybir.dt.float32
    bf16 = mybir.dt.bfloat16
    Exp = mybir.ActivationFunctionType.Exp

    nDK = D // P       # k-tiles over model dim
    nH = E // P        # heads on this core
    nSC = S // CW      # 512-wide s-chunks
    nST = S // P       # 128-wide s-tiles
    TPC = CW // P      # s-tiles per chunk (4)
    SCALE = 1.0 / math.sqrt(P)
    causal = mask_mode == "causal"

    nc = bacc.Bacc("TRN2", target_bir_lowering=False, debug=False)

    xT = nc.dram_tensor("xT", [D, S], bf16, kind="ExternalInput").ap()
    wqT = nc.dram_tensor("wqT", [D, E], bf16, kind="ExternalInput").ap()
    wkT = nc.dram_tensor("wkT", [D, E], bf16, kind="ExternalInput").ap()
    wvT = nc.dram_tensor("wvT", [D, E], bf16, kind="ExternalInput").ap()
    woT = nc.dram_tensor("woT", [E, D], bf16, kind="ExternalInput").ap()
    cs = nc.dram_tensor("cs", [P, S], f32, kind="ExternalInput").ap()
    maskd = nc.dram_tensor("maskd", [P, P], f32, kind="ExternalInput").ap()
    if mask_mode == "generic":
        maskT = nc.dram_tensor("maskT", [S, S], bf16, kind="ExternalInput").ap()
    y = nc.dram_tensor("y", [S, D], f32, kind="ExternalOutput").ap()

    with tile.TileContext(nc) as tc, ExitStack() as ctx:
        const = ctx.enter_context(tc.tile_pool(name="const", bufs=1))
        tp = ctx.enter_context(tc.tile_pool(name="tmp", bufs=2))
        expp = ctx.enter_context(tc.tile_pool(name="expp", bufs=6))
        sbB = ctx.enter_context(tc.tile_pool(name="sbB", bufs=2))
        yp = ctx.enter_context(tc.tile_pool(name="yp", bufs=4))
        psA = ctx.enter_context(tc.tile_pool(name="psA", bufs=4, space="PSUM"))
        psB = ctx.enter_context(tc.tile_pool(name="psB", bufs=2, space="PSUM"))
        psD = ctx.enter_context(tc.tile_pool(name="psD", bufs=2, space="PSUM"))

        # ---- persistent tiles --------------------------------------------
        qt = const.tile([P, nH, S], bf16)    # rotated Q^T  (re rows 0-63)
        kt = const.tile([P, nH, S], bf16)    # rotated K^T
        v = const.tile([P, nST, E], bf16)    # V [s within tile, stile, e]
        outT = const.tile([P, nH, S], bf16)  # attention out^T per head
        cs_t = const.tile([P, S], f32)       # rows 0-63 cos^T, 64-127 sin^T
        md = const.tile([P, P], f32)         # diag mask block^T / SCALE
        ones_col = const.tile([P, 1], bf16)
        ones_row = const.tile([1, P], bf16)

        nc.vector.memset(ones_col, 1.0)
        nc.vector.memset(ones_row, 1.0)
        nc.sync.dma_start(out=cs_t, in_=cs)
        nc.sync.dma_start(out=md, in_=maskd)

        def rope(ps, dst, col):
            """ps: [128, CW] psum raw projection (re rows 0-63, im 64-127).
            dst: [128, CW] bf16 sbuf destination slice. col: s-slice."""
            re, im = ps[0:64, :], ps[64:128, :]
            cosv, sinv = cs_t[0:64, col], cs_t[64:128, col]
            t1 = tp.tile([64, CW], f32, tag="t1", name="t1")
            t2 = tp.tile([64, CW], f32, tag="t2", name="t2")
            nc.vector.tensor_mul(t1, re, cosv)
            nc.vector.tensor_mul(t2, im, sinv)
            nc.vector.tensor_sub(dst[0:64, :], t1, t2)
            t3 = tp.tile([64, CW], f32, tag="t1", name="t3")
            t4 = tp.tile([64, CW], f32, tag="t2", name="t4")
            nc.vector.tensor_mul(t3, re, sinv)
            nc.vector.tensor_mul(t4, im, cosv)
            nc.vector.tensor_add(dst[64:128, :], t3, t4)

        # ---- phase 1: Q^T / K^T / V projections --------------------------
        with tc.tile_pool(name="xw", bufs=1) as xtp, \
             tc.tile_pool(name="wz", bufs=3) as wpool:
            # DMA ordering: wq first (1st matmul's stationary), then x^T in
            # dk-quarter tiles on the OTHER HWDGE ring (scalar) so the PE can
            # start as soon as wq + the first x quarter land, streaming
            # behind the remaining transfers.
            nXQ = 4
            nKQ = nDK // nXQ
            wdmas = {}
            for proj, wdram in enumerate((wqT, wkT, wvT)):
                wdmas[proj] = wdram.rearrange("(dk p) e -> p dk e", p=P)

            wts_q = []
            for kh in range(2):
                wt = wpool.tile([P, nDK // 2, E], bf16, tag="w", name="wt")
                nc.sync.dma_start(
                    out=wt,
                    in_=wdmas[0][:, kh * (nDK // 2):(kh + 1) * (nDK // 2), :])
                wts_q.append(wt)

            xts = []
            for kq in range(nXQ):
                xt = xtp.tile([P, nKQ, S], bf16, tag=f"xt{kq}", name="xt")
                nc.scalar.dma_start(
                    out=xt,
                    in_=xT.rearrange("(dk p) s -> p dk s", p=P)[
                        :, kq * nKQ:(kq + 1) * nKQ, :])
                xts.append(xt)

            def xslice(dk, ssl):
                return xts[dk // nKQ][:, dk % nKQ, ssl]

            for proj, (wdram, dest) in enumerate(((wqT, qt), (wkT, kt), (wvT, v))):
                if proj == 0:
                    wts = wts_q
                else:
                    wts = []
                    for kh in range(2):
                        wt = wpool.tile([P, nDK // 2, E], bf16, tag="w", name="wt")
                        nc.sync.dma_start(
                            out=wt,
                            in_=wdmas[proj][:, kh * (nDK // 2):(kh + 1) * (nDK // 2), :])
                        wts.append(wt)

                def wslice(dk, esl):
                    return wts[dk // (nDK // 2)][:, dk % (nDK // 2), esl]

                if proj < 2:
                    for h in range(nH):
                        esl = slice(h * P, (h + 1) * P)
                        for sc in range(nSC):
                            col = slice(sc * CW, (sc + 1) * CW)
                            ps = psA.tile([P, CW], f32, tag="psA", name="ps_qk")
                            for dk in range(nDK):
                                nc.tensor.matmul(
                                    ps, wslice(dk, esl), xslice(dk, col),
                                    start=(dk == 0), stop=(dk == nDK - 1))
                            rope(ps, dest[:, h, col], col)
                else:
                    for st in range(nST):
                        ssl = slice(st * P, (st + 1) * P)
                        ps = psA.tile([P, CW], f32, tag="psA", name="ps_v")
                        for dk in range(nDK):
                            nc.tensor.matmul(
                                ps[:, 0:E], xslice(dk, ssl),
                                wslice(dk, slice(0, E)),
                                start=(dk == 0), stop=(dk == nDK - 1))
                        nc.scalar.copy(v[:, st, :], ps[:, 0:E])

        # ---- late pool (reuses xt/w space) -------------------------------
        late = ctx.enter_context(tc.tile_pool(name="late", bufs=1))
        wo_t = late.tile([P, nH, D], bf16)
        nc.sync.dma_start(out=wo_t, in_=woT.rearrange("(h p) d -> p h d", p=P))

        # ---- phase 2: attention ------------------------------------------
        for c in range(nSC):
            qcol = slice(c * CW, (c + 1) * CW)
            if mask_mode == "generic":
                mk = late.tile([P, nST, CW], bf16, tag="mk", name="mk", bufs=2)
                nc.sync.dma_start(
                    out=mk,
                    in_=maskT.rearrange("(j p) q -> p j q", p=P)[:, :, qcol])
            jmax = TPC * c + TPC - 1 if causal else nST - 1
            # Interleave head pairs: per-j the dependency chain
            # scores->mask->exp->denom/PV has ~1us of cross-engine latency
            # vs ~0.65us of PE work; two independent heads in flight keep
            # the PE saturated.
            hgroups = [list(range(hp, min(hp + 2, nH))) for hp in range(0, nH, 2)]
            for hg in hgroups:
                ps_o = {h: psB.tile([P, CW], f32, tag="psB", name="ps_o")
                        for h in hg}
                ps_d = {h: psD.tile([1, CW], f32, tag="psD", name="ps_d")
                        for h in hg}
                for j in range(jmax + 1):
                    o = max(0, j - TPC * c) * P if causal else 0
                    for h in hg:
                        ps_s = psA.tile([P, CW], f32, tag="psA", name="ps_s")
                        nc.tensor.matmul(
                            ps_s[:, o:], kt[:, h, j * P:(j + 1) * P],
                            qt[:, h, c * CW + o:(c + 1) * CW],
                            start=True, stop=True)
                        if causal:
                            if j >= TPC * c:
                                nc.vector.tensor_add(
                                    ps_s[:, o:o + P], ps_s[:, o:o + P], md)
                        elif mask_mode == "generic":
                            nc.vector.tensor_add(ps_s, ps_s, mk[:, j, :])
                        es = expp.tile([P, CW], bf16, tag="es", name="es")
                        nc.scalar.activation(es[:, o:], ps_s[:, o:], Exp,
                                             scale=SCALE)
                        nc.tensor.matmul(ps_d[h][:, o:], ones_col, es[:, o:],
                                         start=(j == 0), stop=(j == jmax))
                        nc.tensor.matmul(ps_o[h][:, o:],
                                         v[:, j, h * P:(h + 1) * P],
                                         es[:, o:], start=(j == 0),
                                         stop=(j == jmax))
                for h in hg:
                    # normalize: out^T[:, sq] *= 1/denom[sq].  Copy denom out
                    # fast (frees the psD bank), broadcast, and take the
                    # reciprocal on all 128 partitions (a [1,CW] reciprocal
                    # runs on a single DVE lane: ~3.3us vs ~0.6us).
                    dd = tp.tile([1, CW], f32, tag="rr", name="dd")
                    nc.scalar.copy(dd, ps_d[h])
                    bc = sbB.tile([P, CW], f32, tag="bc", name="bc")
                    nc.gpsimd.partition_broadcast(out_ap=bc, in_ap=dd)
                    bcr = sbB.tile([P, CW], f32, tag="bcr", name="bcr")
                    nc.vector.reciprocal(bcr, bc)
                    nc.vector.tensor_mul(outT[:, h, qcol], ps_o[h], bcr)

        # ---- phase 3: output projection (row-parallel partial) -----------
        nDC = D // CW
        for m in range(nST):
            for dc in range(nDC):
                ps_y = psA.tile([P, CW], f32, tag="psA", name="ps_y")
                for h in range(nH):
                    nc.tensor.matmul(
                        ps_y, outT[:, h, m * P:(m + 1) * P],
                        wo_t[:, h, dc * CW:(dc + 1) * CW],
                        start=(h == 0), stop=(h == nH - 1))
                yo = yp.tile([P, CW], f32, tag="yo", name="yo")
                nc.scalar.copy(yo, ps_y)
                nc.sync.dma_start(
                    out=y[m * P:(m + 1) * P, dc * CW:(dc + 1) * CW], in_=yo)

    nc.compile()
    return nc


def _get_built(mask_mode, S, D, E):
    key = (mask_mode, S, D, E)
    if key not in _built_cache:
        _built_cache[key] = _build(S=S, D=D, E=E, mask_mode=mask_mode)
    return _built_cache[key]


def _classify_mask(mask):
    S = mask.shape[0]
    if not mask.any():
        return "none"
    causal = np.where(np.triu(np.ones((S, S), dtype=bool), k=1),
                      np.float32(-1e9), np.float32(0.0))
    if np.array_equal(mask, causal):
        return "causal"
    return "generic"


def make_in_maps(x, wq, wk, wv, wo, freqs_cos, freqs_sin, mask, n_cores=8):
    """Host-side sharding + layout prep. Returns (in_maps, mask_mode, meta)."""
    bf = ml_dtypes.bfloat16
    x = np.asarray(x, np.float32)
    B, S, D = x.shape
    groups = n_cores // B
    E = D // groups
    nH = E // P
    scale = 1.0 / math.sqrt(P)

    mask = np.asarray(mask, np.float32)
    mode = _classify_mask(mask)

    fc = np.asarray(freqs_cos, np.float32)
    fs = np.asarray(freqs_sin, np.float32)
    cs = np.concatenate(
        [np.ascontiguousarray(fc.T), np.ascontiguousarray(fs.T)], axis=0
    ).astype(np.float32)                      # [128, S]
    maskd = np.ascontiguousarray(mask[0:P, 0:P].T / scale).astype(np.float32)

    # per-head deinterleave: head-local columns [0,2,...,126,1,3,...,127]
    perm1 = np.concatenate([np.arange(0, P, 2), np.arange(1, P, 2)])
    permE = np.concatenate([h * P + perm1 for h in range(nH)])

    wqT_f = np.asarray(wq, np.float32).T      # [D, D]
    wkT_f = np.asarray(wk, np.float32).T
    wvT_f = np.asarray(wv, np.float32).T
    woT_f = np.asarray(wo, np.float32).T      # [E_total, D]

    if mode == "generic":
        maskT_bf = np.ascontiguousarray(mask.T / scale).astype(bf)

    xT_b = [np.ascontiguousarray(x[b].T).astype(bf) for b in range(B)]

    in_maps = []
    for c in range(n_cores):
        b, g = divmod(c, groups)
        es = slice(g * E, (g + 1) * E)
        m = {
            "xT": xT_b[b],
            "wqT": np.ascontiguousarray(wqT_f[:, es][:, permE]).astype(bf),
            "wkT": np.ascontiguousarray(wkT_f[:, es][:, permE]).astype(bf),
            "wvT": np.ascontiguousarray(wvT_f[:, es]).astype(bf),
            "woT": np.ascontiguousarray(woT_f[es, :]).astype(bf),
            "cs": cs,
            "maskd": maskd,
        }
        if mode == "generic":
            m["maskT"] = maskT_bf
        in_maps.append(m)
    return in_maps, mode, (B, S, D, E, groups)


def kernel(x, wq, wk, wv, wo, freqs_cos, freqs_sin, mask, start_pos=0, **_):
    from concourse.bass_utils import run_bass_kernel_spmd

    in_maps, mode, (B, S, D, E, groups) = make_in_maps(
        x, wq, wk, wv, wo, freqs_cos, freqs_sin, mask)
    nc = _get_built(mode, S, D, E)
    res = run_bass_kernel_spmd(nc, in_maps, core_ids=list(range(len(in_maps))))
    parts = [r["y"] for r in res.results]
    out = np.stack(
        [np.sum(parts[b * groups:(b + 1) * groups], axis=0) for b in range(B)]
    ).astype(np.float32)
    return out


# revision 15
# speedup vs baseline: 1.0745x; 1.0745x over previous
"""Trainium2 Bass kernel: multi-head causal attention with RoPE (LLaMA-style).

Problem: y = Attention(x) with B=2, S=2048, D=2048, H=16 heads, HD=128,
torch-Linear convention (y = x @ W.T), interleaved-rope, additive mask.

Sharding (8 NeuronCores): batch (2) x head-groups (4) grid.  Core c handles
batch b = c // 4 and heads 4g..4g+3 where g = c % 4 (tensor parallel:
wq/wk/wv column-parallel, wo row-parallel).  Each core returns a partial
y contribution [S, D]; the host sums the 4 partials per batch.

Layout strategy (no on-chip transposes anywhere):
  - Host pre-transposes: xT [D,S], wqT/wkT/wvT [D,E], woT [E,D].
  - Q^T,K^T computed directly in [hd, s] layout (hd = partitions) with the
    head-dim DEINTERLEAVED (rows 0-63 = even/"re" dims, 64-127 = odd/"im")
    by permuting wq/wk columns on the host; RoPE is then plain 64-partition
    elementwise ops.  The permutation is invisible to Q.K^T contraction.
  - scores are computed TRANSPOSED [sk, sq] so softmax-denominators come
    from a ones-matmul (column sums) and exp(scores)^T feeds the PV matmul
    directly as the moving operand: P^T never materializes.
  - attention out falls out as out^T [hd, sq] = exactly the stationary
    layout the wo row-parallel matmul wants.
Matmul inputs are bf16 (fp32 PSUM accumulation); softmax runs in fp32.
"""

import math
from contextlib import ExitStack

import numpy as np
import ml_dtypes

P = 128          # partitions / head dim
CW = 512         # s-chunk width (one PSUM bank of fp32)

_built_cache = {}


def _build(*, S, D, E, mask_mode):
    """Build + compile the SPMD Bass program for one core's shard.

    S: sequence length, D: model dim, E: head-columns per core (nH*128).
    mask_mode: 'causal' (use diag block + skip upper triangle),
               'none' (no mask, full attention),
               'generic' (arbitrary additive mask, applied everywhere).
    """
    import concourse.bacc as bacc
    import concourse.mybir as mybir
    import concourse.tile as tile

    f32 = mybir.dt.float32
    bf16 = mybir.dt.bfloat16
    Exp = mybir.ActivationFunctionType.Exp

    nDK = D // P       # k-tiles over model dim
    nH = E // P        # heads on this core
    nSC = S // CW      # 512-wide s-chunks
    nST = S // P       # 128-wide s-tiles
    TPC = CW // P      # s-tiles per chunk (4)
    SCALE = 1.0 / math.sqrt(P)
    causal = mask_mode == "causal"

    nc = bacc.Bacc("TRN2", target_bir_lowering=False, debug=False)

    xT = nc.dram_tensor("xT", [D, S], bf16, kind="ExternalInput").ap()
    wqT = nc.dram_tensor("wqT", [D, E], bf16, kind="ExternalInput").ap()
    wkT = nc.dram_tensor("wkT", [D, E], bf16, kind="ExternalInput").ap()
    wvT = nc.dram_tensor("wvT", [D, E], bf16, kind="ExternalInput").ap()
    woT = nc.dram_tensor("woT", [E, D], bf16, kind="ExternalInput").ap()
    cs = nc.dram_tensor("cs", [P, S], f32, kind="ExternalInput").ap()
    maskd = nc.dram_tensor("maskd", [P, P], f32, kind="ExternalInput").ap()
    if mask_mode == "generic":
        maskT = nc.dram_tensor("maskT", [S, S], bf16, kind="ExternalInput").ap()
    y = nc.dram_tensor("y", [S, D], f32, kind="ExternalOutput").ap()

    with tile.TileContext(nc) as tc, ExitStack() as ctx:
        const = ctx.enter_context(tc.tile_pool(name="const", bufs=1))
        tp = ctx.enter_context(tc.tile_pool(name="tmp", bufs=2))
        expp = ctx.enter_context(tc.tile_pool(name="expp", bufs=6))
        sbB = ctx.enter_context(tc.tile_pool(name="sbB", bufs=2))
        yp = ctx.enter_context(tc.tile_pool(name="yp", bufs=3))
        psA = ctx.enter_context(tc.tile_pool(name="psA", bufs=4, space="PSUM"))
        psB = ctx.enter_context(tc.tile_pool(name="psB", bufs=2, space="PSUM"))
        psD = ctx.enter_context(tc.tile_pool(name="psD", bufs=2, space="PSUM"))

        # ---- persistent tiles --------------------------------------------
        qt = const.tile([P, nH, S], bf16)    # rotated Q^T  (re rows 0-63)
        kt = const.tile([P, nH, S], bf16)    # rotated K^T
        v = const.tile([P, nST, E], bf16)    # V [s within tile, stile, e]
        outT = const.tile([P, nH, S], bf16)  # attention out^T per head
        cs_t = const.tile([P, S], f32)       # rows 0-63 cos^T, 64-127 sin^T
        md = const.tile([P, P], f32)         # diag mask block^T / SCALE
        ones_col = const.tile([P, 1], bf16)
        ones_row = const.tile([1, P], bf16)

        nc.vector.memset(ones_col, 1.0)
        nc.vector.memset(ones_row, 1.0)

        def rope(ps, dst, col):
            """ps: [128, CW] psum raw projection (re rows 0-63, im 64-127).
            dst: [128, CW] bf16 sbuf destination slice. col: s-slice.
            NB the walrus verifier requires all SBUF *inputs* of a
            tensor-tensor op to share a start partition; PSUM inputs and the
            output are exempt, so each product takes one aligned SBUF input
            and the combines read base-0 tiles."""
            re, im = ps[0:64, :], ps[64:128, :]
            cosv, sinv = cs_t[0:64, col], cs_t[64:128, col]
            t1 = tp.tile([64, CW], f32, tag="t1", name="t1")
            t2 = tp.tile([64, CW], f32, tag="t2", name="t2")
            nc.vector.tensor_mul(t1, re, cosv)
            nc.vector.tensor_mul(t2, im, sinv)
            nc.vector.tensor_sub(dst[0:64, :], t1, t2)
            t3 = tp.tile([64, CW], f32, tag="t1", name="t3")
            t4 = tp.tile([64, CW], f32, tag="t2", name="t4")
            nc.vector.tensor_mul(t3, re, sinv)
            nc.vector.tensor_mul(t4, im, cosv)
            nc.vector.tensor_add(dst[64:128, :], t3, t4)

        # ---- phase 1: Q^T / K^T / V projections --------------------------
        with tc.tile_pool(name="xw", bufs=1) as xtp, \
             tc.tile_pool(name="wz", bufs=3) as wpool:
            # DMA ordering: wq first on the sync HWDGE ring (it is the first
            # matmul's stationary operand), x^T in dk-eighth tiles on the
            # scalar HWDGE ring, so the PE starts as soon as wq + the first
            # x piece land and streams behind the remaining transfers.
            nXQ = min(8, nDK)
            nKQ = nDK // nXQ
            wdmas = {}
            for proj, wdram in enumerate((wqT, wkT, wvT)):
                wdmas[proj] = wdram.rearrange("(dk p) e -> p dk e", p=P)

            wts_q = []
            for kh in range(2):
                wt = wpool.tile([P, nDK // 2, E], bf16, tag="w", name="wt")
                nc.sync.dma_start(
                    out=wt,
                    in_=wdmas[0][:, kh * (nDK // 2):(kh + 1) * (nDK // 2), :])
                wts_q.append(wt)
            nc.sync.dma_start(out=cs_t, in_=cs)
            nc.sync.dma_start(out=md, in_=maskd)

            xts = []
            for kq in range(nXQ):
                xt = xtp.tile([P, nKQ, S], bf16, tag=f"xt{kq}", name="xt")
                nc.scalar.dma_start(
                    out=xt,
                    in_=xT.rearrange("(dk p) s -> p dk s", p=P)[
                        :, kq * nKQ:(kq + 1) * nKQ, :])
                xts.append(xt)

            def xslice(dk, ssl):
                return xts[dk // nKQ][:, dk % nKQ, ssl]

            for proj, (wdram, dest) in enumerate(((wqT, qt), (wkT, kt), (wvT, v))):
                if proj == 0:
                    wts = wts_q
                else:
                    wts = []
                    for kh in range(2):
                        wt = wpool.tile([P, nDK // 2, E], bf16, tag="w", name="wt")
                        nc.sync.dma_start(
                            out=wt,
                            in_=wdmas[proj][:, kh * (nDK // 2):(kh + 1) * (nDK // 2), :])
                        wts.append(wt)

                def wslice(dk, esl):
                    return wts[dk // (nDK // 2)][:, dk % (nDK // 2), esl]

                if proj < 2:
                    for h in range(nH):
                        esl = slice(h * P, (h + 1) * P)
                        for sc in range(nSC):
                            col = slice(sc * CW, (sc + 1) * CW)
                            ps = psA.tile([P, CW], f32, tag="psA", name="ps_qk")
                            for dk in range(nDK):
                                nc.tensor.matmul(
                                    ps, wslice(dk, esl), xslice(dk, col),
                                    start=(dk == 0), stop=(dk == nDK - 1))
                            rope(ps, dest[:, h, col], col)
                else:
                    for st in range(nST):
                        ssl = slice(st * P, (st + 1) * P)
                        ps = psA.tile([P, CW], f32, tag="psA", name="ps_v")
                        for dk in range(nDK):
                            nc.tensor.matmul(
                                ps[:, 0:E], xslice(dk, ssl),
                                wslice(dk, slice(0, E)),
                                start=(dk == 0), stop=(dk == nDK - 1))
                        nc.scalar.copy(v[:, st, :], ps[:, 0:E])

        # ---- late pool (reuses xt/w space) -------------------------------
        late = ctx.enter_context(tc.tile_pool(name="late", bufs=1))
        wo_t = late.tile([P, nH, D], bf16)
        nc.sync.dma_start(out=wo_t, in_=woT.rearrange("(h p) d -> p h d", p=P))

        # ---- phase 2: attention ------------------------------------------
        for c in range(nSC):
            qcol = slice(c * CW, (c + 1) * CW)
            if mask_mode == "generic":
                mk = late.tile([P, nST, CW], bf16, tag="mk", name="mk", bufs=2)
                nc.sync.dma_start(
                    out=mk,
                    in_=maskT.rearrange("(j p) q -> p j q", p=P)[:, :, qcol])
            jmax = TPC * c + TPC - 1 if causal else nST - 1
            # Interleave head pairs: per-j the dependency chain
            # scores->mask->exp->denom/PV has ~1us of cross-engine latency
            # vs ~0.65us of PE work; two independent heads in flight keep
            # the PE saturated.
            hgroups = [list(range(hp, min(hp + 2, nH))) for hp in range(0, nH, 2)]
            for hg in hgroups:
                ps_o = {h: psB.tile([P, CW], f32, tag="psB", name="ps_o")
                        for h in hg}
                ps_d = {h: psD.tile([1, CW], f32, tag="psD", name="ps_d")
                        for h in hg}
                for j in range(jmax + 1):
                    o = max(0, j - TPC * c) * P if causal else 0
                    for h in hg:
                        ps_s = psA.tile([P, CW], f32, tag="psA", name="ps_s")
                        nc.tensor.matmul(
                            ps_s[:, o:], kt[:, h, j * P:(j + 1) * P],
                            qt[:, h, c * CW + o:(c + 1) * CW],
                            start=True, stop=True)
                        if causal:
                            if j >= TPC * c:
                                nc.vector.tensor_add(
                                    ps_s[:, o:o + P], ps_s[:, o:o + P], md)
                        elif mask_mode == "generic":
                            nc.vector.tensor_add(ps_s, ps_s, mk[:, j, :])
                        es = expp.tile([P, CW], bf16, tag="es", name="es")
                        nc.scalar.activation(es[:, o:], ps_s[:, o:], Exp,
                                             scale=SCALE)
                        nc.tensor.matmul(ps_d[h][:, o:], ones_col, es[:, o:],
                                         start=(j == 0), stop=(j == jmax))
                        nc.tensor.matmul(ps_o[h][:, o:],
                                         v[:, j, h * P:(h + 1) * P],
                                         es[:, o:], start=(j == 0),
                                         stop=(j == jmax))
                for h in hg:
                    # Normalize out^T[:, sq] by 1/denom[sq].  Both PSUM banks
                    # are freed by quick ACT copies (so the next head pair's
                    # accumulations start immediately); the broadcast /
                    # reciprocal / multiply then run entirely in SBUF.
                    dd = tp.tile([1, CW], f32, tag="rr", name="dd")
                    nc.scalar.copy(dd, ps_d[h])
                    ou = expp.tile([P, CW], bf16, tag="ou", name="ou", bufs=3)
                    nc.scalar.copy(ou, ps_o[h])
                    bc = sbB.tile([P, CW], f32, tag="bc", name="bc")
                    nc.gpsimd.partition_broadcast(out_ap=bc, in_ap=dd)
                    bcr = sbB.tile([P, CW], f32, tag="bcr", name="bcr")
                    nc.vector.reciprocal(bcr, bc)
                    nc.vector.tensor_mul(outT[:, h, qcol], ou, bcr)

        # ---- phase 3: output projection (row-parallel partial) -----------
        nDC = D // CW
        for m in range(nST):
            for dc in range(nDC):
                ps_y = psA.tile([P, CW], f32, tag="psA", name="ps_y")
                for h in range(nH):
                    nc.tensor.matmul(
                        ps_y, outT[:, h, m * P:(m + 1) * P],
                        wo_t[:, h, dc * CW:(dc + 1) * CW],
                        start=(h == 0), stop=(h == nH - 1))
                yo = yp.tile([P, CW], f32, tag="yo", name="yo")
                nc.scalar.copy(yo, ps_y)
                nc.sync.dma_start(
                    out=y[m * P:(m + 1) * P, dc * CW:(dc + 1) * CW], in_=yo)

    nc.compile()
    return nc


def _get_built(mask_mode, S, D, E):
    key = (mask_mode, S, D, E)
    if key not in _built_cache:
        _built_cache[key] = _build(S=S, D=D, E=E, mask_mode=mask_mode)
    return _built_cache[key]


def _classify_mask(mask):
    S = mask.shape[0]
    if not mask.any():
        return "none"
    causal = np.where(np.triu(np.ones((S, S), dtype=bool), k=1),
                      np.float32(-1e9), np.float32(0.0))
    if np.array_equal(mask, causal):
        return "causal"
    return "generic"


def make_in_maps(x, wq, wk, wv, wo, freqs_cos, freqs_sin, mask, n_cores=8):
    """Host-side sharding + layout prep. Returns (in_maps, mask_mode, meta)."""
    bf = ml_dtypes.bfloat16
    x = np.asarray(x, np.float32)
    B, S, D = x.shape
    groups = n_cores // B
    E = D // groups
    nH = E // P
    scale = 1.0 / math.sqrt(P)

    mask = np.asarray(mask, np.float32)
    mode = _classify_mask(mask)

    fc = np.asarray(freqs_cos, np.float32)
    fs = np.asarray(freqs_sin, np.float32)
    cs = np.concatenate(
        [np.ascontiguousarray(fc.T), np.ascontiguousarray(fs.T)], axis=0
    ).astype(np.float32)                      # [128, S]
    maskd = np.ascontiguousarray(mask[0:P, 0:P].T / scale).astype(np.float32)

    # per-head deinterleave: head-local columns [0,2,...,126,1,3,...,127]
    perm1 = np.concatenate([np.arange(0, P, 2), np.arange(1, P, 2)])
    permE = np.concatenate([h * P + perm1 for h in range(nH)])

    wqT_f = np.asarray(wq, np.float32).T      # [D, D]
    wkT_f = np.asarray(wk, np.float32).T
    wvT_f = np.asarray(wv, np.float32).T
    woT_f = np.asarray(wo, np.float32).T      # [E_total, D]

    if mode == "generic":
        maskT_bf = np.ascontiguousarray(mask.T / scale).astype(bf)

    xT_b = [np.ascontiguousarray(x[b].T).astype(bf) for b in range(B)]

    in_maps = []
    for c in range(n_cores):
        b, g = divmod(c, groups)
        es = slice(g * E, (g + 1) * E)
        m = {
            "xT": xT_b[b],
            "wqT": np.ascontiguousarray(wqT_f[:, es][:, permE]).astype(bf),
            "wkT": np.ascontiguousarray(wkT_f[:, es][:, permE]).astype(bf),
            "wvT": np.ascontiguousarray(wvT_f[:, es]).astype(bf),
            "woT": np.ascontiguousarray(woT_f[es, :]).astype(bf),
            "cs": cs,
            "maskd": maskd,
        }
        if mode == "generic":
            m["maskT"] = maskT_bf
        in_maps.append(m)
    return in_maps, mode, (B, S, D, E, groups)


def kernel(x, wq, wk, wv, wo, freqs_cos, freqs_sin, mask, start_pos=0, **_):
    from concourse.bass_utils import run_bass_kernel_spmd

    in_maps, mode, (B, S, D, E, groups) = make_in_maps(
        x, wq, wk, wv, wo, freqs_cos, freqs_sin, mask)
    nc = _get_built(mode, S, D, E)
    res = run_bass_kernel_spmd(nc, in_maps, core_ids=list(range(len(in_maps))))
    parts = [r["y"] for r in res.results]
    out = np.stack(
        [np.sum(parts[b * groups:(b + 1) * groups], axis=0) for b in range(B)]
    ).astype(np.float32)
    return out


# revision 19
# speedup vs baseline: 1.0956x; 1.0196x over previous
"""Trainium2 Bass kernel: multi-head causal attention with RoPE (LLaMA-style).

Problem: y = Attention(x) with B=2, S=2048, D=2048, H=16 heads, HD=128,
torch-Linear convention (y = x @ W.T), interleaved-rope, additive mask.

Sharding (8 NeuronCores): batch (2) x head-groups (4) grid.  Core c handles
batch b = c // 4 and heads 4g..4g+3 where g = c % 4 (tensor parallel:
wq/wk/wv column-parallel, wo row-parallel).  Each core returns a partial
y contribution [S, D]; the host sums the 4 partials per batch.

Layout strategy (no on-chip transposes anywhere):
  - Host pre-transposes: xT [D,S], wqT/wkT/wvT [D,E], woT [E,D].
  - Q^T,K^T computed directly in [hd, s] layout (hd = partitions) with the
    head-dim DEINTERLEAVED (rows 0-63 = even/"re" dims, 64-127 = odd/"im")
    by permuting wq/wk columns on the host; RoPE is then plain 64-partition
    elementwise ops.  The permutation is invisible to Q.K^T contraction.
  - scores are computed TRANSPOSED [sk, sq] so softmax-denominators come
    from a ones-matmul (column sums) and exp(scores)^T feeds the PV matmul
    directly as the moving operand: P^T never materializes.
  - attention out falls out as out^T [hd, sq] = exactly the stationary
    layout the wo row-parallel matmul wants.
Matmul inputs are bf16 (fp32 PSUM accumulation); softmax runs in fp32.
"""

import math
from contextlib import ExitStack

import numpy as np
import ml_dtypes

P = 128          # partitions / head dim
CW = 512         # s-chunk width (one PSUM bank of fp32)

_built_cache = {}


def _build(*, S, D, E, mask_mode):
    """Build + compile the SPMD Bass program for one core's shard.

    S: sequence length, D: model dim, E: head-columns per core (nH*128).
    mask_mode: 'causal' (use diag block + skip upper triangle),
               'none' (no mask, full attention),
               'generic' (arbitrary additive mask, applied everywhere).
    """
    import concourse.bacc as bacc
    import concourse.mybir as mybir
    import concourse.tile as tile

    f32 = mybir.dt.float32
    bf16 = mybir.dt.bfloat16
    Exp = mybir.ActivationFunctionType.Exp

    nDK = D // P       # k-tiles over model dim
    nH = E // P        # heads on this core
    nSC = S // CW      # 512-wide s-chunks
    nST = S // P       # 128-wide s-tiles
    TPC = CW // P      # s-tiles per chunk (4)
    SCALE = 1.0 / math.sqrt(P)
    causal = mask_mode == "causal"

    nc = bacc.Bacc("TRN2", target_bir_lowering=False, debug=False)

    xT = nc.dram_tensor("xT", [D, S], bf16, kind="ExternalInput").ap()
    wqT = nc.dram_tensor("wqT", [D, E], bf16, kind="ExternalInput").ap()
    wkT = nc.dram_tensor("wkT", [D, E], bf16, kind="ExternalInput").ap()
    wvT = nc.dram_tensor("wvT", [D, E], bf16, kind="ExternalInput").ap()
    woT = nc.dram_tensor("woT", [E, D], bf16, kind="ExternalInput").ap()
    cs = nc.dram_tensor("cs", [P, S], f32, kind="ExternalInput").ap()
    maskd = nc.dram_tensor("maskd", [P, P], bf16, kind="ExternalInput").ap()
    if mask_mode == "generic":
        maskT = nc.dram_tensor("maskT", [S, S], bf16, kind="ExternalInput").ap()
    y = nc.dram_tensor("y", [S, D], f32, kind="ExternalOutput").ap()

    with tile.TileContext(nc) as tc, ExitStack() as ctx:
        const = ctx.enter_context(tc.tile_pool(name="const", bufs=1))
        tp = ctx.enter_context(tc.tile_pool(name="tmp", bufs=2))
        expp = ctx.enter_context(tc.tile_pool(name="expp", bufs=6))
        sbB = ctx.enter_context(tc.tile_pool(name="sbB", bufs=2))
        yp = ctx.enter_context(tc.tile_pool(name="yp", bufs=3))
        psA = ctx.enter_context(tc.tile_pool(name="psA", bufs=5, space="PSUM"))
        psB = ctx.enter_context(tc.tile_pool(name="psB", bufs=2, space="PSUM"))
        psD = ctx.enter_context(tc.tile_pool(name="psD", bufs=1, space="PSUM"))

        # ---- persistent tiles --------------------------------------------
        qt = const.tile([P, nH, S], bf16)    # rotated Q^T  (re rows 0-63)
        kt = const.tile([P, nH, S], bf16)    # rotated K^T
        v = const.tile([P, nST, E], bf16)    # V [s within tile, stile, e]
        outT = const.tile([P, nH, S], bf16)  # attention out^T per head
        cs_t = const.tile([P, S], f32)       # rows 0-63 cos^T, 64-127 sin^T
        md = const.tile([P, P], bf16)        # exp(diag mask block^T): 0/1 for causal
        ones_col = const.tile([P, 1], bf16)
        ones_row = const.tile([1, P], bf16)

        nc.vector.memset(ones_col, 1.0)
        nc.vector.memset(ones_row, 1.0)

        def rope(ps, dst, col):
            """ps: [128, CW] psum raw projection (re rows 0-63, im 64-127).
            dst: [128, CW] bf16 sbuf destination slice. col: s-slice.
            NB the walrus verifier requires all SBUF *inputs* of a
            tensor-tensor op to share a start partition; PSUM inputs and the
            output are exempt, so each product takes one aligned SBUF input
            and the combines read base-0 tiles."""
            re, im = ps[0:64, :], ps[64:128, :]
            cosv, sinv = cs_t[0:64, col], cs_t[64:128, col]
            t1 = tp.tile([64, CW], f32, tag="t1", name="t1")
            t2 = tp.tile([64, CW], f32, tag="t2", name="t2")
            nc.vector.tensor_mul(t1, re, cosv)
            nc.vector.tensor_mul(t2, im, sinv)
            nc.vector.tensor_sub(dst[0:64, :], t1, t2)
            t3 = tp.tile([64, CW], f32, tag="t1", name="t3")
            t4 = tp.tile([64, CW], f32, tag="t2", name="t4")
            nc.vector.tensor_mul(t3, re, sinv)
            nc.vector.tensor_mul(t4, im, cosv)
            nc.vector.tensor_add(dst[64:128, :], t3, t4)

        # ---- phase 1: Q^T / K^T / V projections --------------------------
        with tc.tile_pool(name="xw", bufs=1) as xtp, \
             tc.tile_pool(name="wz", bufs=3) as wpool:
            # DMA ordering: wq first on the sync HWDGE ring (it is the first
            # matmul's stationary operand), x^T in dk-eighth tiles on the
            # scalar HWDGE ring, so the PE starts as soon as wq + the first
            # x piece land and streams behind the remaining transfers.
            nXQ = min(8, nDK)
            nKQ = nDK // nXQ
            wdmas = {}
            for proj, wdram in enumerate((wqT, wkT, wvT)):
                wdmas[proj] = wdram.rearrange("(dk p) e -> p dk e", p=P)

            wts_q = []
            for kh in range(2):
                wt = wpool.tile([P, nDK // 2, E], bf16, tag="w", name="wt")
                nc.sync.dma_start(
                    out=wt,
                    in_=wdmas[0][:, kh * (nDK // 2):(kh + 1) * (nDK // 2), :])
                wts_q.append(wt)
            nc.sync.dma_start(out=cs_t, in_=cs)
            nc.sync.dma_start(out=md, in_=maskd)

            xts = []
            for kq in range(nXQ):
                xt = xtp.tile([P, nKQ, S], bf16, tag=f"xt{kq}", name="xt")
                nc.scalar.dma_start(
                    out=xt,
                    in_=xT.rearrange("(dk p) s -> p dk s", p=P)[
                        :, kq * nKQ:(kq + 1) * nKQ, :])
                xts.append(xt)

            def xslice(dk, ssl):
                return xts[dk // nKQ][:, dk % nKQ, ssl]

            for proj, (wdram, dest) in enumerate(((wqT, qt), (wkT, kt), (wvT, v))):
                if proj == 0:
                    wts = wts_q
                else:
                    wts = []
                    for kh in range(2):
                        wt = wpool.tile([P, nDK // 2, E], bf16, tag="w", name="wt")
                        nc.sync.dma_start(
                            out=wt,
                            in_=wdmas[proj][:, kh * (nDK // 2):(kh + 1) * (nDK // 2), :])
                        wts.append(wt)

                def wslice(dk, esl):
                    return wts[dk // (nDK // 2)][:, dk % (nDK // 2), esl]

                if proj < 2:
                    for h in range(nH):
                        esl = slice(h * P, (h + 1) * P)
                        for sc in range(nSC):
                            col = slice(sc * CW, (sc + 1) * CW)
                            ps = psA.tile([P, CW], f32, tag="psA", name="ps_qk")
                            for dk in range(nDK):
                                nc.tensor.matmul(
                                    ps, wslice(dk, esl), xslice(dk, col),
                                    start=(dk == 0), stop=(dk == nDK - 1))
                            rope(ps, dest[:, h, col], col)
                else:
                    for st in range(nST):
                        ssl = slice(st * P, (st + 1) * P)
                        ps = psA.tile([P, CW], f32, tag="psA", name="ps_v")
                        for dk in range(nDK):
                            nc.tensor.matmul(
                                ps[:, 0:E], xslice(dk, ssl),
                                wslice(dk, slice(0, E)),
                                start=(dk == 0), stop=(dk == nDK - 1))
                        nc.scalar.copy(v[:, st, :], ps[:, 0:E])

        # ---- late pool (reuses xt/w space) -------------------------------
        late = ctx.enter_context(tc.tile_pool(name="late", bufs=1))
        wo_t = late.tile([P, nH, D], bf16)
        nc.sync.dma_start(out=wo_t, in_=woT.rearrange("(h p) d -> p h d", p=P))

        # ---- phase 2: attention ------------------------------------------
        # Masking happens OFF the PSUM critical chain: es := exp(scale*scores)
        # is multiplied by exp(mask) in SBUF (exact 0/1 for causal), so each
        # scores PSUM bank is held only for matmul -> exp.  The scores+exp
        # for iteration j+2 are emitted ahead of the denominator/PV matmuls
        # of iteration j to cover the cross-engine exp latency.
        for c in range(nSC):
            qcol = slice(c * CW, (c + 1) * CW)
            if mask_mode == "generic":
                mk = late.tile([P, nST, CW], bf16, tag="mk", name="mk", bufs=2)
                nc.sync.dma_start(
                    out=mk,
                    in_=maskT.rearrange("(j p) q -> p j q", p=P)[:, :, qcol])
            jmax = TPC * c + TPC - 1 if causal else nST - 1
            for h in range(nH):
                ps_o = psB.tile([P, CW], f32, tag="psB", name="ps_o")
                ps_d = psD.tile([1, CW], f32, tag="psD", name="ps_d")
                ess = {}

                def emit_scores(j):
                    o = max(0, j - TPC * c) * P if causal else 0
                    ps_s = psA.tile([P, CW], f32, tag="psA", name="ps_s")
                    nc.tensor.matmul(
                        ps_s[:, o:], kt[:, h, j * P:(j + 1) * P],
                        qt[:, h, c * CW + o:(c + 1) * CW],
                        start=True, stop=True)
                    es = expp.tile([P, CW], bf16, tag="es", name="es")
                    nc.scalar.activation(es[:, o:], ps_s[:, o:], Exp,
                                         scale=SCALE)
                    if causal:
                        if j >= TPC * c:
                            nc.vector.tensor_mul(
                                es[:, o:o + P], es[:, o:o + P], md)
                    elif mask_mode == "generic":
                        nc.vector.tensor_mul(es, es, mk[:, j, :])
                    ess[j] = (es, o)

                emit_scores(0)
                if jmax >= 1:
                    emit_scores(1)
                for j in range(jmax + 1):
                    if j + 2 <= jmax:
                        emit_scores(j + 2)
                    es, o = ess.pop(j)
                    nc.tensor.matmul(ps_d[:, o:], ones_col, es[:, o:],
                                     start=(j == 0), stop=(j == jmax))
                    nc.tensor.matmul(ps_o[:, o:], v[:, j, h * P:(h + 1) * P],
                                     es[:, o:], start=(j == 0), stop=(j == jmax))
                # Normalize out^T[:, sq] by 1/denom[sq].  Both PSUM banks are
                # freed by quick ACT copies; broadcast / reciprocal / multiply
                # run entirely in SBUF.
                dd = tp.tile([1, CW], f32, tag="rr", name="dd")
                nc.scalar.copy(dd, ps_d)
                ou = expp.tile([P, CW], bf16, tag="ou", name="ou", bufs=3)
                nc.scalar.copy(ou, ps_o)
                bc = sbB.tile([P, CW], f32, tag="bc", name="bc")
                nc.gpsimd.partition_broadcast(out_ap=bc, in_ap=dd)
                bcr = sbB.tile([P, CW], f32, tag="bcr", name="bcr")
                nc.vector.reciprocal(bcr, bc)
                nc.vector.tensor_mul(outT[:, h, qcol], ou, bcr)

        # ---- phase 3: output projection (row-parallel partial) -----------
        nDC = D // CW
        for m in range(nST):
            for dc in range(nDC):
                ps_y = psA.tile([P, CW], f32, tag="psA", name="ps_y")
                for h in range(nH):
                    nc.tensor.matmul(
                        ps_y, outT[:, h, m * P:(m + 1) * P],
                        wo_t[:, h, dc * CW:(dc + 1) * CW],
                        start=(h == 0), stop=(h == nH - 1))
                yo = yp.tile([P, CW], f32, tag="yo", name="yo")
                nc.scalar.copy(yo, ps_y)
                nc.sync.dma_start(
                    out=y[m * P:(m + 1) * P, dc * CW:(dc + 1) * CW], in_=yo)

    nc.compile()
    return nc


def _get_built(mask_mode, S, D, E):
    key = (mask_mode, S, D, E)
    if key not in _built_cache:
        _built_cache[key] = _build(S=S, D=D, E=E, mask_mode=mask_mode)
    return _built_cache[key]


def _classify_mask(mask):
    S = mask.shape[0]
    if not mask.any():
        return "none"
    causal = np.where(np.triu(np.ones((S, S), dtype=bool), k=1),
                      np.float32(-1e9), np.float32(0.0))
    if np.array_equal(mask, causal):
        return "causal"
    return "generic"


def make_in_maps(x, wq, wk, wv, wo, freqs_cos, freqs_sin, mask, n_cores=8):
    """Host-side sharding + layout prep. Returns (in_maps, mask_mode, meta)."""
    bf = ml_dtypes.bfloat16
    x = np.asarray(x, np.float32)
    B, S, D = x.shape
    groups = n_cores // B
    E = D // groups
    nH = E // P
    scale = 1.0 / math.sqrt(P)

    mask = np.asarray(mask, np.float32)
    mode = _classify_mask(mask)

    fc = np.asarray(freqs_cos, np.float32)
    fs = np.asarray(freqs_sin, np.float32)
    cs = np.concatenate(
        [np.ascontiguousarray(fc.T), np.ascontiguousarray(fs.T)], axis=0
    ).astype(np.float32)                      # [128, S]
    # masking is applied multiplicatively on exp(scores): exp(mask) — exact
    # 0/1 for the causal -1e9/0 mask
    maskd = np.exp(np.ascontiguousarray(mask[0:P, 0:P].T)).astype(bf)

    # per-head deinterleave: head-local columns [0,2,...,126,1,3,...,127]
    perm1 = np.concatenate([np.arange(0, P, 2), np.arange(1, P, 2)])
    permE = np.concatenate([h * P + perm1 for h in range(nH)])

    wqT_f = np.asarray(wq, np.float32).T      # [D, D]
    wkT_f = np.asarray(wk, np.float32).T
    wvT_f = np.asarray(wv, np.float32).T
    woT_f = np.asarray(wo, np.float32).T      # [E_total, D]

    if mode == "generic":
        maskT_bf = np.exp(np.ascontiguousarray(mask.T)).astype(bf)

    xT_b = [np.ascontiguousarray(x[b].T).astype(bf) for b in range(B)]

    in_maps = []
    for c in range(n_cores):
        b, g = divmod(c, groups)
        es = slice(g * E, (g + 1) * E)
        m = {
            "xT": xT_b[b],
            "wqT": np.ascontiguousarray(wqT_f[:, es][:, permE]).astype(bf),
            "wkT": np.ascontiguousarray(wkT_f[:, es][:, permE]).astype(bf),
            "wvT": np.ascontiguousarray(wvT_f[:, es]).astype(bf),
            "woT": np.ascontiguousarray(woT_f[es, :]).astype(bf),
            "cs": cs,
            "maskd": maskd,
        }
        if mode == "generic":
            m["maskT"] = maskT_bf
        in_maps.append(m)
    return in_maps, mode, (B, S, D, E, groups)


def kernel(x, wq, wk, wv, wo, freqs_cos, freqs_sin, mask, start_pos=0, **_):
    from concourse.bass_utils import run_bass_kernel_spmd

    in_maps, mode, (B, S, D, E, groups) = make_in_maps(
        x, wq, wk, wv, wo, freqs_cos, freqs_sin, mask)
    nc = _get_built(mode, S, D, E)
    res = run_bass_kernel_spmd(nc, in_maps, core_ids=list(range(len(in_maps))))
    parts = [r["y"] for r in res.results]
    out = np.stack(
        [np.sum(parts[b * groups:(b + 1) * groups], axis=0) for b in range(B)]
    ).astype(np.float32)
    return out


# revision 20
# speedup vs baseline: 1.1381x; 1.0388x over previous
"""Trainium2 Bass kernel: multi-head causal attention with RoPE (LLaMA-style).

Problem: y = Attention(x) with B=2, S=2048, D=2048, H=16 heads, HD=128,
torch-Linear convention (y = x @ W.T), interleaved-rope, additive mask.

Sharding (8 NeuronCores): batch (2) x head-groups (4) grid.  Core c handles
batch b = c // 4 and heads 4g..4g+3 where g = c % 4 (tensor parallel:
wq/wk/wv column-parallel, wo row-parallel).  Each core returns a partial
y contribution [S, D]; the host sums the 4 partials per batch.

Layout strategy (no on-chip transposes anywhere):
  - Host pre-transposes: xT [D,S], wqT/wkT/wvT [D,E], woT [E,D].
  - Q^T,K^T computed directly in [hd, s] layout (hd = partitions) with the
    head-dim DEINTERLEAVED (rows 0-63 = even/"re" dims, 64-127 = odd/"im")
    by permuting wq/wk columns on the host; RoPE is then plain 64-partition
    elementwise ops.  The permutation is invisible to Q.K^T contraction.
  - scores are computed TRANSPOSED [sk, sq] so softmax-denominators come
    from a ones-matmul (column sums) and exp(scores)^T feeds the PV matmul
    directly as the moving operand: P^T never materializes.
  - attention out falls out as out^T [hd, sq] = exactly the stationary
    layout the wo row-parallel matmul wants.
Matmul inputs are bf16 (fp32 PSUM accumulation); softmax runs in fp32.
"""

import math
from contextlib import ExitStack

import numpy as np
import ml_dtypes

P = 128          # partitions / head dim
CW = 512         # s-chunk width (one PSUM bank of fp32)

_built_cache = {}


def _build(*, S, D, E, mask_mode):
    """Build + compile the SPMD Bass program for one core's shard.

    S: sequence length, D: model dim, E: head-columns per core (nH*128).
    mask_mode: 'causal' (use diag block + skip upper triangle),
               'none' (no mask, full attention),
               'generic' (arbitrary additive mask, applied everywhere).
    """
    import concourse.bacc as bacc
    import concourse.mybir as mybir
    import concourse.tile as tile

    f32 = mybir.dt.float32
    bf16 = mybir.dt.bfloat16
    Exp = mybir.ActivationFunctionType.Exp

    nDK = D // P       # k-tiles over model dim
    nH = E // P        # heads on this core
    nSC = S // CW      # 512-wide s-chunks
    nST = S // P       # 128-wide s-tiles
    TPC = CW // P      # s-tiles per chunk (4)
    SCALE = 1.0 / math.sqrt(P)
    causal = mask_mode == "causal"

    nc = bacc.Bacc("TRN2", target_bir_lowering=False, debug=False)

    xT = nc.dram_tensor("xT", [D, S], bf16, kind="ExternalInput").ap()
    wqT = nc.dram_tensor("wqT", [D, E], bf16, kind="ExternalInput").ap()
    wkT = nc.dram_tensor("wkT", [D, E], bf16, kind="ExternalInput").ap()
    wvT = nc.dram_tensor("wvT", [D, E], bf16, kind="ExternalInput").ap()
    woT = nc.dram_tensor("woT", [E, D], bf16, kind="ExternalInput").ap()
    cs = nc.dram_tensor("cs", [P, S], f32, kind="ExternalInput").ap()
    maskd = nc.dram_tensor("maskd", [P, P], bf16, kind="ExternalInput").ap()
    if mask_mode == "generic":
        maskT = nc.dram_tensor("maskT", [S, S], bf16, kind="ExternalInput").ap()
    y = nc.dram_tensor("y", [S, D], f32, kind="ExternalOutput").ap()

    with tile.TileContext(nc) as tc, ExitStack() as ctx:
        const = ctx.enter_context(tc.tile_pool(name="const", bufs=1))
        tp = ctx.enter_context(tc.tile_pool(name="tmp", bufs=2))
        expp = ctx.enter_context(tc.tile_pool(name="expp", bufs=6))
        sbB = ctx.enter_context(tc.tile_pool(name="sbB", bufs=2))
        yp = ctx.enter_context(tc.tile_pool(name="yp", bufs=3))
        psA = ctx.enter_context(tc.tile_pool(name="psA", bufs=5, space="PSUM"))
        psB = ctx.enter_context(tc.tile_pool(name="psB", bufs=2, space="PSUM"))
        psD = ctx.enter_context(tc.tile_pool(name="psD", bufs=1, space="PSUM"))

        # ---- persistent tiles --------------------------------------------
        qt = const.tile([P, nH, S], bf16)    # rotated Q^T  (re rows 0-63)
        kt = const.tile([P, nH, S], bf16)    # rotated K^T
        v = const.tile([P, nST, E], bf16)    # V [s within tile, stile, e]
        outT = const.tile([P, nH, S], bf16)  # attention out^T per head
        cs_t = const.tile([P, S], f32)       # rows 0-63 cos^T, 64-127 sin^T
        md = const.tile([P, P], bf16)        # exp(diag mask block^T): 0/1 for causal
        ones_col = const.tile([P, 1], bf16)
        ones_row = const.tile([1, P], bf16)

        nc.vector.memset(ones_col, 1.0)
        nc.vector.memset(ones_row, 1.0)

        def rope(ps, dst, col):
            """ps: [128, CW] psum raw projection (re rows 0-63, im 64-127).
            dst: [128, CW] bf16 sbuf destination slice. col: s-slice.
            NB the walrus verifier requires all SBUF *inputs* of a
            tensor-tensor op to share a start partition; PSUM inputs and the
            output are exempt, so each product takes one aligned SBUF input
            and the combines read base-0 tiles."""
            re, im = ps[0:64, :], ps[64:128, :]
            cosv, sinv = cs_t[0:64, col], cs_t[64:128, col]
            t1 = tp.tile([64, CW], f32, tag="t1", name="t1")
            t2 = tp.tile([64, CW], f32, tag="t2", name="t2")
            nc.vector.tensor_mul(t1, re, cosv)
            nc.vector.tensor_mul(t2, im, sinv)
            nc.vector.tensor_sub(dst[0:64, :], t1, t2)
            t3 = tp.tile([64, CW], f32, tag="t1", name="t3")
            t4 = tp.tile([64, CW], f32, tag="t2", name="t4")
            nc.vector.tensor_mul(t3, re, sinv)
            nc.vector.tensor_mul(t4, im, cosv)
            nc.vector.tensor_add(dst[64:128, :], t3, t4)

        # ---- phase 1: Q^T / K^T / V projections --------------------------
        with tc.tile_pool(name="xw", bufs=1) as xtp, \
             tc.tile_pool(name="wz", bufs=3) as wpool:
            # DMA ordering: wq first on the sync HWDGE ring (it is the first
            # matmul's stationary operand), x^T in dk-eighth tiles on the
            # scalar HWDGE ring, so the PE starts as soon as wq + the first
            # x piece land and streams behind the remaining transfers.
            nXQ = min(8, nDK)
            nKQ = nDK // nXQ
            wdmas = {}
            for proj, wdram in enumerate((wqT, wkT, wvT)):
                wdmas[proj] = wdram.rearrange("(dk p) e -> p dk e", p=P)

            wts_q = []
            for kh in range(2):
                wt = wpool.tile([P, nDK // 2, E], bf16, tag="w", name="wt")
                nc.sync.dma_start(
                    out=wt,
                    in_=wdmas[0][:, kh * (nDK // 2):(kh + 1) * (nDK // 2), :])
                wts_q.append(wt)
            nc.sync.dma_start(out=cs_t, in_=cs)
            nc.sync.dma_start(out=md, in_=maskd)

            xts = []
            for kq in range(nXQ):
                xt = xtp.tile([P, nKQ, S], bf16, tag=f"xt{kq}", name="xt")
                nc.scalar.dma_start(
                    out=xt,
                    in_=xT.rearrange("(dk p) s -> p dk s", p=P)[
                        :, kq * nKQ:(kq + 1) * nKQ, :])
                xts.append(xt)

            def xslice(dk, ssl):
                return xts[dk // nKQ][:, dk % nKQ, ssl]

            for proj, (wdram, dest) in enumerate(((wqT, qt), (wkT, kt), (wvT, v))):
                if proj == 0:
                    wts = wts_q
                else:
                    wts = []
                    for kh in range(2):
                        wt = wpool.tile([P, nDK // 2, E], bf16, tag="w", name="wt")
                        nc.sync.dma_start(
                            out=wt,
                            in_=wdmas[proj][:, kh * (nDK // 2):(kh + 1) * (nDK // 2), :])
                        wts.append(wt)

                def wslice(dk, esl):
                    return wts[dk // (nDK // 2)][:, dk % (nDK // 2), esl]

                if proj < 2:
                    for h in range(nH):
                        esl = slice(h * P, (h + 1) * P)
                        for sc in range(nSC):
                            col = slice(sc * CW, (sc + 1) * CW)
                            ps = psA.tile([P, CW], f32, tag="psA", name="ps_qk")
                            for dk in range(nDK):
                                nc.tensor.matmul(
                                    ps, wslice(dk, esl), xslice(dk, col),
                                    start=(dk == 0), stop=(dk == nDK - 1))
                            rope(ps, dest[:, h, col], col)
                else:
                    for st in range(nST):
                        ssl = slice(st * P, (st + 1) * P)
                        ps = psA.tile([P, CW], f32, tag="psA", name="ps_v")
                        for dk in range(nDK):
                            nc.tensor.matmul(
                                ps[:, 0:E], xslice(dk, ssl),
                                wslice(dk, slice(0, E)),
                                start=(dk == 0), stop=(dk == nDK - 1))
                        nc.scalar.copy(v[:, st, :], ps[:, 0:E])

        # ---- late pool (reuses xt/w space) -------------------------------
        late = ctx.enter_context(tc.tile_pool(name="late", bufs=1))
        wo_t = late.tile([P, nH, D], bf16)
        nc.sync.dma_start(out=wo_t, in_=woT.rearrange("(h p) d -> p h d", p=P))

        # ---- phase 2: attention ------------------------------------------
        # Masking happens OFF the PSUM critical chain: es := exp(scale*scores)
        # is multiplied by exp(mask) in SBUF (exact 0/1 for causal), so each
        # scores PSUM bank is held only for matmul -> exp.  The scores+exp
        # for iteration j+2 are emitted ahead of the denominator/PV matmuls
        # of iteration j to cover the cross-engine exp latency.
        for c in range(nSC):
            qcol = slice(c * CW, (c + 1) * CW)
            if mask_mode == "generic":
                mk = late.tile([P, nST, CW], bf16, tag="mk", name="mk", bufs=2)
                nc.sync.dma_start(
                    out=mk,
                    in_=maskT.rearrange("(j p) q -> p j q", p=P)[:, :, qcol])
            jmax = TPC * c + TPC - 1 if causal else nST - 1
            for h in range(nH):
                ps_o = psB.tile([P, CW], f32, tag="psB", name="ps_o")
                ps_d = psD.tile([1, CW], f32, tag="psD", name="ps_d")
                ess = {}

                def emit_scores(j):
                    o = max(0, j - TPC * c) * P if causal else 0
                    ps_s = psA.tile([P, CW], f32, tag="psA", name="ps_s")
                    nc.tensor.matmul(
                        ps_s[:, o:], kt[:, h, j * P:(j + 1) * P],
                        qt[:, h, c * CW + o:(c + 1) * CW],
                        start=True, stop=True)
                    es = expp.tile([P, CW], bf16, tag="es", name="es")
                    nc.scalar.activation(es[:, o:], ps_s[:, o:], Exp,
                                         scale=SCALE)
                    if causal:
                        if j >= TPC * c:
                            nc.vector.tensor_mul(
                                es[:, o:o + P], es[:, o:o + P], md)
                    elif mask_mode == "generic":
                        nc.vector.tensor_mul(es, es, mk[:, j, :])
                    ess[j] = (es, o)

                for jj in range(min(3, jmax + 1)):
                    emit_scores(jj)
                for j in range(jmax + 1):
                    if j + 3 <= jmax:
                        emit_scores(j + 3)
                    es, o = ess.pop(j)
                    nc.tensor.matmul(ps_d[:, o:], ones_col, es[:, o:],
                                     start=(j == 0), stop=(j == jmax))
                    nc.tensor.matmul(ps_o[:, o:], v[:, j, h * P:(h + 1) * P],
                                     es[:, o:], start=(j == 0), stop=(j == jmax))
                # Normalize out^T[:, sq] by 1/denom[sq].  Both PSUM banks are
                # freed by quick ACT copies; broadcast / reciprocal / multiply
                # run entirely in SBUF.
                dd = tp.tile([1, CW], f32, tag="rr", name="dd")
                nc.vector.tensor_copy(dd, ps_d)
                ou = expp.tile([P, CW], bf16, tag="ou", name="ou", bufs=3)
                nc.vector.tensor_copy(ou, ps_o)
                bc = sbB.tile([P, CW], f32, tag="bc", name="bc")
                nc.gpsimd.partition_broadcast(out_ap=bc, in_ap=dd)
                bcr = sbB.tile([P, CW], f32, tag="bcr", name="bcr")
                nc.vector.reciprocal_approx_fast(out=bcr, in_=bc)
                nc.vector.tensor_mul(outT[:, h, qcol], ou, bcr)

        # ---- phase 3: output projection (row-parallel partial) -----------
        nDC = D // CW
        for m in range(nST):
            for dc in range(nDC):
                ps_y = psA.tile([P, CW], f32, tag="psA", name="ps_y")
                for h in range(nH):
                    nc.tensor.matmul(
                        ps_y, outT[:, h, m * P:(m + 1) * P],
                        wo_t[:, h, dc * CW:(dc + 1) * CW],
                        start=(h == 0), stop=(h == nH - 1))
                yo = yp.tile([P, CW], f32, tag="yo", name="yo")
                nc.vector.tensor_copy(yo, ps_y)
                nc.sync.dma_start(
                    out=y[m * P:(m + 1) * P, dc * CW:(dc + 1) * CW], in_=yo)

    nc.compile()
    return nc


def _get_built(mask_mode, S, D, E):
    key = (mask_mode, S, D, E)
    if key not in _built_cache:
        _built_cache[key] = _build(S=S, D=D, E=E, mask_mode=mask_mode)
    return _built_cache[key]


def _classify_mask(mask):
    S = mask.shape[0]
    if not mask.any():
        return "none"
    causal = np.where(np.triu(np.ones((S, S), dtype=bool), k=1),
                      np.float32(-1e9), np.float32(0.0))
    if np.array_equal(mask, causal):
        return "causal"
    return "generic"


def make_in_maps(x, wq, wk, wv, wo, freqs_cos, freqs_sin, mask, n_cores=8):
    """Host-side sharding + layout prep. Returns (in_maps, mask_mode, meta)."""
    bf = ml_dtypes.bfloat16
    x = np.asarray(x, np.float32)
    B, S, D = x.shape
    groups = n_cores // B
    E = D // groups
    nH = E // P
    scale = 1.0 / math.sqrt(P)

    mask = np.asarray(mask, np.float32)
    mode = _classify_mask(mask)

    fc = np.asarray(freqs_cos, np.float32)
    fs = np.asarray(freqs_sin, np.float32)
    cs = np.concatenate(
        [np.ascontiguousarray(fc.T), np.ascontiguousarray(fs.T)], axis=0
    ).astype(np.float32)                      # [128, S]
    # masking is applied multiplicatively on exp(scores): exp(mask) — exact
    # 0/1 for the causal -1e9/0 mask
    maskd = np.exp(np.ascontiguousarray(mask[0:P, 0:P].T)).astype(bf)

    # per-head deinterleave: head-local columns [0,2,...,126,1,3,...,127]
    perm1 = np.concatenate([np.arange(0, P, 2), np.arange(1, P, 2)])
    permE = np.concatenate([h * P + perm1 for h in range(nH)])

    wqT_f = np.asarray(wq, np.float32).T      # [D, D]
    wkT_f = np.asarray(wk, np.float32).T
    wvT_f = np.asarray(wv, np.float32).T
    woT_f = np.asarray(wo, np.float32).T      # [E_total, D]

    if mode == "generic":
        maskT_bf = np.exp(np.ascontiguousarray(mask.T)).astype(bf)

    xT_b = [np.ascontiguousarray(x[b].T).astype(bf) for b in range(B)]

    in_maps = []
    for c in range(n_cores):
        b, g = divmod(c, groups)
        es = slice(g * E, (g + 1) * E)
        m = {
            "xT": xT_b[b],
            "wqT": np.ascontiguousarray(wqT_f[:, es][:, permE]).astype(bf),
            "wkT": np.ascontiguousarray(wkT_f[:, es][:, permE]).astype(bf),
            "wvT": np.ascontiguousarray(wvT_f[:, es]).astype(bf),
            "woT": np.ascontiguousarray(woT_f[es, :]).astype(bf),
            "cs": cs,
            "maskd": maskd,
        }
        if mode == "generic":
            m["maskT"] = maskT_bf
        in_maps.append(m)
    return in_maps, mode, (B, S, D, E, groups)


def kernel(x, wq, wk, wv, wo, freqs_cos, freqs_sin, mask, start_pos=0, **_):
    from concourse.bass_utils import run_bass_kernel_spmd

    in_maps, mode, (B, S, D, E, groups) = make_in_maps(
        x, wq, wk, wv, wo, freqs_cos, freqs_sin, mask)
    nc = _get_built(mode, S, D, E)
    res = run_bass_kernel_spmd(nc, in_maps, core_ids=list(range(len(in_maps))))
    parts = [r["y"] for r in res.results]
    out = np.stack(
        [np.sum(parts[b * groups:(b + 1) * groups], axis=0) for b in range(B)]
    ).astype(np.float32)
    return out


# revision 21
# speedup vs baseline: 1.1620x; 1.0210x over previous
"""Trainium2 Bass kernel: multi-head causal attention with RoPE (LLaMA-style).

Problem: y = Attention(x) with B=2, S=2048, D=2048, H=16 heads, HD=128,
torch-Linear convention (y = x @ W.T), interleaved-rope, additive mask.

Sharding (8 NeuronCores): batch (2) x head-groups (4) grid.  Core c handles
batch b = c // 4 and heads 4g..4g+3 where g = c % 4 (tensor parallel:
wq/wk/wv column-parallel, wo row-parallel).  Each core returns a partial
y contribution [S, D]; the host sums the 4 partials per batch.

Layout strategy (no on-chip transposes anywhere):
  - Host pre-transposes: xT [D,S], wqT/wkT/wvT [D,E], woT [E,D].
  - Q^T,K^T computed directly in [hd, s] layout (hd = partitions) with the
    head-dim DEINTERLEAVED (rows 0-63 = even/"re" dims, 64-127 = odd/"im")
    by permuting wq/wk columns on the host; RoPE is then plain 64-partition
    elementwise ops.  The permutation is invisible to Q.K^T contraction.
  - scores are computed TRANSPOSED [sk, sq] so softmax-denominators come
    from a ones-matmul (column sums) and exp(scores)^T feeds the PV matmul
    directly as the moving operand: P^T never materializes.
  - attention out falls out as out^T [hd, sq] = exactly the stationary
    layout the wo row-parallel matmul wants.
Matmul inputs are bf16 (fp32 PSUM accumulation); softmax runs in fp32.
"""

import math
from contextlib import ExitStack

import numpy as np
import ml_dtypes

P = 128          # partitions / head dim
CW = 512         # s-chunk width (one PSUM bank of fp32)

_built_cache = {}


def _build(*, S, D, E, mask_mode):
    """Build + compile the SPMD Bass program for one core's shard.

    S: sequence length, D: model dim, E: head-columns per core (nH*128).
    mask_mode: 'causal' (use diag block + skip upper triangle),
               'none' (no mask, full attention),
               'generic' (arbitrary additive mask, applied everywhere).
    """
    import concourse.bacc as bacc
    import concourse.mybir as mybir
    import concourse.tile as tile

    f32 = mybir.dt.float32
    bf16 = mybir.dt.bfloat16
    Exp = mybir.ActivationFunctionType.Exp

    nDK = D // P       # k-tiles over model dim
    nH = E // P        # heads on this core
    nSC = S // CW      # 512-wide s-chunks
    nST = S // P       # 128-wide s-tiles
    TPC = CW // P      # s-tiles per chunk (4)
    SCALE = 1.0 / math.sqrt(P)
    causal = mask_mode == "causal"

    nc = bacc.Bacc("TRN2", target_bir_lowering=False, debug=False)

    xT = nc.dram_tensor("xT", [D, S], bf16, kind="ExternalInput").ap()
    wqT = nc.dram_tensor("wqT", [D, E], bf16, kind="ExternalInput").ap()
    wkT = nc.dram_tensor("wkT", [D, E], bf16, kind="ExternalInput").ap()
    wvT = nc.dram_tensor("wvT", [D, E], bf16, kind="ExternalInput").ap()
    woT = nc.dram_tensor("woT", [E, D], bf16, kind="ExternalInput").ap()
    cs = nc.dram_tensor("cs", [P, S], f32, kind="ExternalInput").ap()
    maskd = nc.dram_tensor("maskd", [P, P], bf16, kind="ExternalInput").ap()
    if mask_mode == "generic":
        maskT = nc.dram_tensor("maskT", [S, S], bf16, kind="ExternalInput").ap()
    y = nc.dram_tensor("y", [S, D], f32, kind="ExternalOutput").ap()

    with tile.TileContext(nc) as tc, ExitStack() as ctx:
        const = ctx.enter_context(tc.tile_pool(name="const", bufs=1))
        tp = ctx.enter_context(tc.tile_pool(name="tmp", bufs=2))
        expp = ctx.enter_context(tc.tile_pool(name="expp", bufs=6))
        sbB = ctx.enter_context(tc.tile_pool(name="sbB", bufs=2))
        yp = ctx.enter_context(tc.tile_pool(name="yp", bufs=3))
        psA = ctx.enter_context(tc.tile_pool(name="psA", bufs=5, space="PSUM"))
        psB = ctx.enter_context(tc.tile_pool(name="psB", bufs=2, space="PSUM"))
        psD = ctx.enter_context(tc.tile_pool(name="psD", bufs=1, space="PSUM"))

        # ---- persistent tiles --------------------------------------------
        qt = const.tile([P, nH, S], bf16)    # rotated Q^T  (re rows 0-63)
        kt = const.tile([P, nH, S], bf16)    # rotated K^T
        v = const.tile([P, nST, E], bf16)    # V [s within tile, stile, e]
        outT = const.tile([P, nH, S], bf16)  # attention out^T per head
        cs_t = const.tile([P, S], f32)       # rows 0-63 cos^T, 64-127 sin^T
        md = const.tile([P, P], bf16)        # exp(diag mask block^T): 0/1 for causal
        ones_col = const.tile([P, 1], bf16)
        ones_row = const.tile([1, P], bf16)

        nc.vector.memset(ones_col, 1.0)
        nc.vector.memset(ones_row, 1.0)

        def rope(ps, dst, col):
            """ps: [128, CW] psum raw projection (re rows 0-63, im 64-127).
            dst: [128, CW] bf16 sbuf destination slice. col: s-slice.
            NB the walrus verifier requires all SBUF *inputs* of a
            tensor-tensor op to share a start partition; PSUM inputs and the
            output are exempt, so each product takes one aligned SBUF input
            and the combines read base-0 tiles."""
            re, im = ps[0:64, :], ps[64:128, :]
            cosv, sinv = cs_t[0:64, col], cs_t[64:128, col]
            t1 = tp.tile([64, CW], f32, tag="t1", name="t1")
            t2 = tp.tile([64, CW], f32, tag="t2", name="t2")
            nc.vector.tensor_mul(t1, re, cosv)
            nc.vector.tensor_mul(t2, im, sinv)
            nc.vector.tensor_sub(dst[0:64, :], t1, t2)
            t3 = tp.tile([64, CW], f32, tag="t1", name="t3")
            t4 = tp.tile([64, CW], f32, tag="t2", name="t4")
            nc.vector.tensor_mul(t3, re, sinv)
            nc.vector.tensor_mul(t4, im, cosv)
            nc.vector.tensor_add(dst[64:128, :], t3, t4)

        # ---- attention chunk emitter -------------------------------------
        # Masking happens OFF the PSUM critical chain: es := exp(scale*scores)
        # is multiplied by exp(mask) in SBUF (exact 0/1 for causal), so each
        # scores PSUM bank is held only for matmul -> exp.  scores+exp run
        # 3 iterations ahead of the denominator/PV matmuls to cover the
        # cross-engine exp latency.
        def attn_chunk(h, c, mk=None):
            qcol = slice(c * CW, (c + 1) * CW)
            jmax = TPC * c + TPC - 1 if causal else nST - 1
            ps_o = psB.tile([P, CW], f32, tag="psB", name="ps_o")
            ps_d = psD.tile([1, CW], f32, tag="psD", name="ps_d")
            ess = {}

            def emit_scores(j):
                o = max(0, j - TPC * c) * P if causal else 0
                ps_s = psA.tile([P, CW], f32, tag="psA", name="ps_s")
                nc.tensor.matmul(
                    ps_s[:, o:], kt[:, h, j * P:(j + 1) * P],
                    qt[:, h, c * CW + o:(c + 1) * CW],
                    start=True, stop=True)
                es = expp.tile([P, CW], bf16, tag="es", name="es")
                nc.scalar.activation(es[:, o:], ps_s[:, o:], Exp, scale=SCALE)
                if causal:
                    if j >= TPC * c:
                        nc.vector.tensor_mul(
                            es[:, o:o + P], es[:, o:o + P], md)
                elif mask_mode == "generic":
                    nc.vector.tensor_mul(es, es, mk[:, j, :])
                ess[j] = (es, o)

            for jj in range(min(3, jmax + 1)):
                emit_scores(jj)
            for j in range(jmax + 1):
                if j + 3 <= jmax:
                    emit_scores(j + 3)
                es, o = ess.pop(j)
                nc.tensor.matmul(ps_d[:, o:], ones_col, es[:, o:],
                                 start=(j == 0), stop=(j == jmax))
                nc.tensor.matmul(ps_o[:, o:], v[:, j, h * P:(h + 1) * P],
                                 es[:, o:], start=(j == 0), stop=(j == jmax))
            # Normalize out^T[:, sq] by 1/denom[sq].  Both PSUM banks are
            # freed by quick DVE copies; broadcast / reciprocal / multiply
            # run entirely in SBUF.
            dd = tp.tile([1, CW], f32, tag="rr", name="dd")
            nc.vector.tensor_copy(dd, ps_d)
            ou = expp.tile([P, CW], bf16, tag="ou", name="ou", bufs=3)
            nc.vector.tensor_copy(ou, ps_o)
            bc = sbB.tile([P, CW], f32, tag="bc", name="bc")
            nc.gpsimd.partition_broadcast(out_ap=bc, in_ap=dd)
            bcr = sbB.tile([P, CW], f32, tag="bcr", name="bcr")
            nc.vector.reciprocal_approx_fast(out=bcr, in_=bc)
            nc.vector.tensor_mul(outT[:, h, qcol], ou, bcr)

        # ---- projections (V, K, Q0) + interleaved attention --------------
        # Emission order matters: engines execute their queues in program
        # order, so attention for head h (ACT-heavy exp chain) is emitted
        # interleaved with head h+1's Q projection (pure PE work) to keep
        # the PE busy while exp latency drains, and vice versa.
        with tc.tile_pool(name="xw", bufs=1) as xtp, \
             tc.tile_pool(name="wz", bufs=4) as wpool:
            nXQ = min(8, nDK)
            nKQ = nDK // nXQ
            NH2 = nDK // 2
            wdmas = {"q": wqT.rearrange("(dk p) e -> p dk e", p=P),
                     "k": wkT.rearrange("(dk p) e -> p dk e", p=P),
                     "v": wvT.rearrange("(dk p) e -> p dk e", p=P)}

            def load_w(key):
                wts = []
                for kh in range(2):
                    wt = wpool.tile([P, NH2, E], bf16, tag="w", name="wt")
                    nc.sync.dma_start(
                        out=wt, in_=wdmas[key][:, kh * NH2:(kh + 1) * NH2, :])
                    wts.append(wt)
                return wts

            # DMA order: wv first (V projection runs first) on the sync ring,
            # x^T dk-eighths on the scalar ring, then cs/md/wk/wq.
            wv_t = load_w("v")
            xts = []
            for kq in range(nXQ):
                xt = xtp.tile([P, nKQ, S], bf16, tag=f"xt{kq}", name="xt")
                nc.scalar.dma_start(
                    out=xt,
                    in_=xT.rearrange("(dk p) s -> p dk s", p=P)[
                        :, kq * nKQ:(kq + 1) * nKQ, :])
                xts.append(xt)
            nc.sync.dma_start(out=cs_t, in_=cs)
            nc.sync.dma_start(out=md, in_=maskd)
            wk_t = load_w("k")
            wq_t = load_w("q")

            def xslice(dk, ssl):
                return xts[dk // nKQ][:, dk % nKQ, ssl]

            def wslice(wts, dk, esl):
                return wts[dk // NH2][:, dk % NH2, esl]

            # V projection (all heads at once: rhs = all E columns)
            for st in range(nST):
                ssl = slice(st * P, (st + 1) * P)
                ps = psA.tile([P, CW], f32, tag="psA", name="ps_v")
                for dk in range(nDK):
                    nc.tensor.matmul(
                        ps[:, 0:E], xslice(dk, ssl), wslice(wv_t, dk, slice(0, E)),
                        start=(dk == 0), stop=(dk == nDK - 1))
                nc.scalar.copy(v[:, st, :], ps[:, 0:E])

            def qk_group(wts, dest, h, sc):
                esl = slice(h * P, (h + 1) * P)
                col = slice(sc * CW, (sc + 1) * CW)
                ps = psA.tile([P, CW], f32, tag="psA", name="ps_qk")
                for dk in range(nDK):
                    nc.tensor.matmul(
                        ps, wslice(wts, dk, esl), xslice(dk, col),
                        start=(dk == 0), stop=(dk == nDK - 1))
                rope(ps, dest[:, h, col], col)

            for h in range(nH):
                for sc in range(nSC):
                    qk_group(wk_t, kt, h, sc)
            for sc in range(nSC):
                qk_group(wq_t, qt, 0, sc)

            if causal:
                for h in range(nH):
                    for c in range(nSC):
                        attn_chunk(h, c)
                        if h + 1 < nH:
                            qk_group(wq_t, qt, h + 1, c)
            else:
                for h in range(1, nH):
                    for sc in range(nSC):
                        qk_group(wq_t, qt, h, sc)

        # ---- late pool (reuses xt/w space) -------------------------------
        late = ctx.enter_context(tc.tile_pool(name="late", bufs=1))
        wo_t = late.tile([P, nH, D], bf16)
        nc.sync.dma_start(out=wo_t, in_=woT.rearrange("(h p) d -> p h d", p=P))

        if not causal:
            for c in range(nSC):
                mk = None
                if mask_mode == "generic":
                    mk = late.tile([P, nST, CW], bf16, tag="mk", name="mk",
                                   bufs=2)
                    nc.sync.dma_start(
                        out=mk,
                        in_=maskT.rearrange("(j p) q -> p j q", p=P)[
                            :, :, c * CW:(c + 1) * CW])
                for h in range(nH):
                    attn_chunk(h, c, mk=mk)

        # ---- phase 3: output projection (row-parallel partial) -----------
        nDC = D // CW
        for m in range(nST):
            for dc in range(nDC):
                ps_y = psA.tile([P, CW], f32, tag="psA", name="ps_y")
                for h in range(nH):
                    nc.tensor.matmul(
                        ps_y, outT[:, h, m * P:(m + 1) * P],
                        wo_t[:, h, dc * CW:(dc + 1) * CW],
                        start=(h == 0), stop=(h == nH - 1))
                yo = yp.tile([P, CW], f32, tag="yo", name="yo")
                nc.vector.tensor_copy(yo, ps_y)
                nc.sync.dma_start(
                    out=y[m * P:(m + 1) * P, dc * CW:(dc + 1) * CW], in_=yo)

    nc.compile()
    return nc


def _get_built(mask_mode, S, D, E):
    key = (mask_mode, S, D, E)
    if key not in _built_cache:
        _built_cache[key] = _build(S=S, D=D, E=E, mask_mode=mask_mode)
    return _built_cache[key]


def _classify_mask(mask):
    S = mask.shape[0]
    if not mask.any():
        return "none"
    causal = np.where(np.triu(np.ones((S, S), dtype=bool), k=1),
                      np.float32(-1e9), np.float32(0.0))
    if np.array_equal(mask, causal):
        return "causal"
    return "generic"


def make_in_maps(x, wq, wk, wv, wo, freqs_cos, freqs_sin, mask, n_cores=8):
    """Host-side sharding + layout prep. Returns (in_maps, mask_mode, meta)."""
    bf = ml_dtypes.bfloat16
    x = np.asarray(x, np.float32)
    B, S, D = x.shape
    groups = n_cores // B
    E = D // groups
    nH = E // P
    scale = 1.0 / math.sqrt(P)

    mask = np.asarray(mask, np.float32)
    mode = _classify_mask(mask)

    fc = np.asarray(freqs_cos, np.float32)
    fs = np.asarray(freqs_sin, np.float32)
    cs = np.concatenate(
        [np.ascontiguousarray(fc.T), np.ascontiguousarray(fs.T)], axis=0
    ).astype(np.float32)                      # [128, S]
    # masking is applied multiplicatively on exp(scores): exp(mask) — exact
    # 0/1 for the causal -1e9/0 mask
    maskd = np.exp(np.ascontiguousarray(mask[0:P, 0:P].T)).astype(bf)

    # per-head deinterleave: head-local columns [0,2,...,126,1,3,...,127]
    perm1 = np.concatenate([np.arange(0, P, 2), np.arange(1, P, 2)])
    permE = np.concatenate([h * P + perm1 for h in range(nH)])

    wqT_f = np.asarray(wq, np.float32).T      # [D, D]
    wkT_f = np.asarray(wk, np.float32).T
    wvT_f = np.asarray(wv, np.float32).T
    woT_f = np.asarray(wo, np.float32).T      # [E_total, D]

    if mode == "generic":
        maskT_bf = np.exp(np.ascontiguousarray(mask.T)).astype(bf)

    xT_b = [np.ascontiguousarray(x[b].T).astype(bf) for b in range(B)]

    in_maps = []
    for c in range(n_cores):
        b, g = divmod(c, groups)
        es = slice(g * E, (g + 1) * E)
        m = {
            "xT": xT_b[b],
            "wqT": np.ascontiguousarray(wqT_f[:, es][:, permE]).astype(bf),
            "wkT": np.ascontiguousarray(wkT_f[:, es][:, permE]).astype(bf),
            "wvT": np.ascontiguousarray(wvT_f[:, es]).astype(bf),
            "woT": np.ascontiguousarray(woT_f[es, :]).astype(bf),
            "cs": cs,
            "maskd": maskd,
        }
        if mode == "generic":
            m["maskT"] = maskT_bf
        in_maps.append(m)
    return in_maps, mode, (B, S, D, E, groups)


def kernel(x, wq, wk, wv, wo, freqs_cos, freqs_sin, mask, start_pos=0, **_):
    from concourse.bass_utils import run_bass_kernel_spmd

    in_maps, mode, (B, S, D, E, groups) = make_in_maps(
        x, wq, wk, wv, wo, freqs_cos, freqs_sin, mask)
    nc = _get_built(mode, S, D, E)
    res = run_bass_kernel_spmd(nc, in_maps, core_ids=list(range(len(in_maps))))
    parts = [r["y"] for r in res.results]
    out = np.stack(
        [np.sum(parts[b * groups:(b + 1) * groups], axis=0) for b in range(B)]
    ).astype(np.float32)
    return out
